# revision 17
# baseline (speedup 1.0000x reference)
"""CenterLoss kernel for Trainium2 (8 NeuronCores, data-parallel over batch).

loss = mean_i ||x_i - centers[labels_i]||^2   with x [16384,512], centers [4000,512].

Instead of the reference's full [B,C] distance matrix, each core:
  - streams its 2048-row x shard in groups of G row-blocks ([128, G*512]
    tiles, G row-blocks side by side),
  - gathers the matching G*128 center rows per group with a single SWDGE
    dma_gather (ucode-generated descriptors; output lands partition-major,
    exactly matching the x layout), spreading gathers over the SWDGE queues,
  - computes (x - c) in place on the Vector engine, Square-with-accumulate on
    the Scalar engine -> per-partition partial sums,
  - reduces to a [128,1] partial-sum vector that the host combines.

Built on bacc.Bacc so finalize() legalizes the 1-sync-wait-per-instruction
hardware constraint (generate_event_semaphores). A tiny DVE "probe" copy
absorbs the x-DMA wait so the subtract mostly waits on the gather alone.
"""

import numpy as np

try:
    import concourse.bass as bass
except ModuleNotFoundError:  # fallback if the repo isn't on sys.path
    import sys

    sys.path.insert(0, "/opt/trn_rl_repo")
    import concourse.bass as bass

import concourse.bacc as bacc
import concourse.mybir as mybir
import concourse.tile as tile
from concourse.bass_utils import run_bass_kernel_spmd

B, C, D = 16384, 4000, 512
N_CORES = 8
BS = B // N_CORES  # 2048 rows per core
P = 128
NT = BS // P  # 16 row-blocks per core
G = 2  # row-blocks per group (one x DMA + one dma_gather each)
NI = NT // G  # groups per core
GP = G * P  # indices per gather instruction
WCOL = GP // 16  # wrapped index columns per group

_nc_cache = {}


def set_config(g):
    """Adjust the group size (benchmarking experiments)."""
    global G, NI, GP, WCOL
    G = g
    NI = NT // G
    GP = G * P
    WCOL = GP // 16


def build_bass(reps=1, nq=4):
    # reps>1 repeats the computation (benchmarking only); nq = SWDGE queues.
    nc = bacc.Bacc(num_swdge_queues=nq, dynamic_dma_scratch_size=65536)
    x_d = nc.declare_dram_parameter("x", [BS, D], mybir.dt.float32, isOutput=False)
    # wrapped int16 labels: element (p % 16, t*WCOL + k//16) = labels[t*GP + k]
    # replicated across all 128 partitions
    lab_d = nc.declare_dram_parameter(
        "labels16", [P, NI * WCOL], mybir.dt.int16, isOutput=False
    )
    ctr_d = nc.declare_dram_parameter("centers", [C, D], mybir.dt.float32, isOutput=False)
    out_d = nc.declare_dram_parameter("out", [P, 1], mybir.dt.float32, isOutput=True)

    with tile.TileContext(nc) as tc:
        with (
            tc.tile_pool(name="const", bufs=1) as const_pool,
            tc.tile_pool(name="xp", bufs=NI) as xpool,
            tc.tile_pool(name="cp", bufs=NI) as cpool,
            tc.tile_pool(name="sp", bufs=NI) as spool,
            tc.tile_pool(name="pr", bufs=NI) as prpool,
        ):
            lab = const_pool.tile([P, NI * WCOL], mybir.dt.int16)
            nc.sync.dma_start(out=lab[:], in_=lab_d[:])
            ss_all = const_pool.tile([P, NI], mybir.dt.float32)

            for t in range(NI * reps):
                t = t % NI
                xt = xpool.tile([P, G * D], mybir.dt.float32)
                # row-block n of this group lands in columns [n*D, (n+1)*D)
                xin = x_d[t * GP : (t + 1) * GP, :].rearrange("(n p) d -> p n d", p=P)
                nc.sync.dma_start(
                    out=xt[:].rearrange("p (n d) -> p n d", d=D), in_=xin
                )
                ct = cpool.tile([P, G * D], mybir.dt.float32)
                nc.gpsimd.dma_gather(
                    out_ap=ct[:].rearrange("p (n d) -> p n d", d=D),
                    in_ap=ctr_d[:],
                    idxs_ap=lab[:, t * WCOL : (t + 1) * WCOL],
                    num_idxs=GP,
                    num_idxs_reg=GP,
                    elem_size=D,
                    queue_num=t % nq,
                )
                # probe: absorbs the x-DMA wait on the DVE queue so the
                # subtract right after it only carries the gather wait
                pr = prpool.tile([P, 1], mybir.dt.float32)
                nc.vector.tensor_copy(out=pr[:], in_=xt[:, 0:1])
                nc.vector.tensor_sub(xt[:], xt[:], ct[:])  # xt <- x - c
                sq = spool.tile([P, G * D], mybir.dt.float32)
                nc.scalar.activation(
                    out=sq[:],
                    in_=xt[:],
                    func=mybir.ActivationFunctionType.Square,
                    accum_out=ss_all[:, t : t + 1],
                )

            acc = const_pool.tile([P, 1], mybir.dt.float32)
            nc.vector.tensor_reduce(
                out=acc[:], in_=ss_all[:], axis=mybir.AxisListType.X, op=mybir.AluOpType.add
            )
            nc.sync.dma_start(out=out_d[:], in_=acc[:])
    return nc


def wrap_labels(ls):
    """[BS] int -> [P, NI*WCOL] int16 wrapped: (k%16, t*WCOL + k//16) = ls[t*GP+k]."""
    w = ls.reshape(NI, WCOL, 16)  # [t, col, p16]
    w = w.transpose(2, 0, 1).reshape(16, NI * WCOL)  # [p16, t*WCOL+col]
    return np.ascontiguousarray(np.tile(w, (P // 16, 1)).astype(np.int16))


def shard_inputs(x, labels, centers):
    x = np.ascontiguousarray(np.asarray(x), dtype=np.float32)
    labels = np.asarray(labels).astype(np.int64)
    centers = np.ascontiguousarray(np.asarray(centers), dtype=np.float32)
    in_maps = []
    for c in range(N_CORES):
        xs = x[c * BS : (c + 1) * BS]
        ls = labels[c * BS : (c + 1) * BS]
        in_maps.append(
            {
                "x": xs,
                "labels16": wrap_labels(ls),
                "centers": centers,
            }
        )
    return in_maps


def run(x, labels, centers, trace=False, **kwargs):
    if "nc" not in _nc_cache:
        nc = build_bass()
        if not nc.is_finalized():
            nc.finalize()
        _nc_cache["nc"] = nc
    nc = _nc_cache["nc"]
    in_maps = shard_inputs(x, labels, centers)
    res = run_bass_kernel_spmd(nc, in_maps, list(range(N_CORES)), trace=trace, **kwargs)
    total = sum(float(r["out"].astype(np.float64).sum()) for r in res.results)
    return np.float32(total / B), res


def kernel(x, labels, centers):
    out, _ = run(x, labels, centers)
    return out
